# revision 50
# baseline (speedup 1.0000x reference)
"""Batched MoE (dense routing) Trainium2 kernel, v3.

Reference computation (per batch row b):
    alpha = softmax(x @ Wg + bg)                      # (B, E)
    h = relu(x @ W0[e]); h = relu(h @ W1[e]); h = relu(h @ W2[e])
    h3[e] = h @ W3[e]
    y = sum_e alpha[:, e, None] * h3[e]               # (B, 128)

Shapes: B=65536, D=512, E=8, DH=128, DOUT=128.

Strategy: data-parallel shard B across 8 NeuronCores (8192 rows each);
weights replicated.  All matmul operands are bf16 (fp32 PSUM accumulate),
activations stay in transposed [feature, batch] layout so every MLP matmul
streams with free dim 512.

The alpha-weighted combine is folded into the pipeline algebraically:
alpha >= 0, so alpha*relu(z) = relu(alpha*z) and the scale can be applied
to any one layer.  The h2 relu eviction becomes a scalar_tensor_tensor
(relu, then multiply by the UNNORMALIZED exp(logit) row broadcast across
partitions by the otherwise-idle GPSIMD engine's partition_broadcast), and
the final layer ACCUMULATES over all 8 experts in PSUM (start=True only on
the bank's very first write -- it clears has_written for the whole bank).
The v1 per-expert combine (~180us of vector-engine time) disappears; one
scaled eviction per chunk stores the finished output.

Softmax normalization rides the output eviction: the gate matmul uses
4x-replicated Wg columns so exp(logits) lands on 32 partitions, a DVE 32x32
block transpose + free-axis reduce gives batch-partitioned sums, and a
cheap [32,16] reciprocal + 4 crossbar quadrant copies (stream_shuffle)
produce the [128, BB] per-partition 1/sum used by the eviction's
tensor_scalar_mul.  Nothing of this touches the PE or the scalar engine.

The expert loop is software-pipelined with a GLOBAL stage counter that
crosses chunk boundaries (stage s runs L0 of expert-stage s, L1 of s-1, L2
of s-2, L3 of s-3), so the in-order PE sees no bubble at chunk seams: the
next chunk's L0 matmuls separate the last expert's L2 from its L3.  h0/h1
relu evictions are split between the scalar and vector engines to balance
load.  Startup: weight/x DMAs are interleaved by first-use time across the
two HWDGE queues (sync + scalar), w0 lives in per-pair tiles so early
experts do not wait on the full megabyte (tile-granular DMA dependencies),
x is prefetched two chunks ahead, and a dozen dummy matmuls on the
first-to-land gate weights warm the HAM clock throttle before the real
pipeline starts.
"""

import numpy as np

import concourse.bass as bass
import concourse.tile as tile
from concourse import bacc, mybir
from concourse.bass_utils import run_bass_kernel_spmd

B, D, E, DH, DOUT = 65536, 512, 8, 128, 128
N_CORES = 8
B_LOCAL = B // N_CORES          # 8192
NB = 512                        # batch rows per chunk
CHUNKS = B_LOCAL // NB          # 16
DK = D // 128                   # 4 k-tiles over the input dim
P = 128
BB = NB // P                    # 4 batch sub-tiles per chunk
S = CHUNKS * E                  # 128 global expert-stages

F32 = mybir.dt.float32
F32R = mybir.dt.float32r
BF16 = mybir.dt.bfloat16

# how many of the 8 per-chunk h1 evictions go to the vector engine
# (the rest go to the scalar engine, which also has h0 + the gate exp)
H1_ON_VECTOR = 2

_CACHE = {}


def _build():
    if "nc" in _CACHE:
        return _CACHE["nc"]

    nc = bacc.Bacc("TRN2", target_bir_lowering=False, debug=False,
                   num_devices=N_CORES)

    xt_ap = nc.dram_tensor("xt", [D, B_LOCAL], BF16, kind="ExternalInput").ap()
    w0_ap = nc.dram_tensor("w0", [P, E, DK, DH], BF16, kind="ExternalInput").ap()
    w1_ap = nc.dram_tensor("w1", [P, E, DH], BF16, kind="ExternalInput").ap()
    w2_ap = nc.dram_tensor("w2", [P, E, DH], BF16, kind="ExternalInput").ap()
    w3_ap = nc.dram_tensor("w3", [P, E, DH], BF16, kind="ExternalInput").ap()
    wg_ap = nc.dram_tensor("wg", [P, DK, 32], BF16, kind="ExternalInput").ap()
    bg_ap = nc.dram_tensor("bg", [32, 1], F32, kind="ExternalInput").ap()
    y_ap = nc.dram_tensor("y", [B_LOCAL, DOUT], F32, kind="ExternalOutput").ap()

    with tile.TileContext(nc) as tc:
        with (
            tc.tile_pool(name="weights", bufs=1) as wpool,
            tc.tile_pool(name="xt", bufs=4) as xpool,
            tc.tile_pool(name="h", bufs=9) as hpool,
            tc.tile_pool(name="soft", bufs=3) as spool,
            tc.tile_pool(name="stg", bufs=4) as stgpool,
            tc.tile_pool(name="ezbn", bufs=14) as bpool,
            tc.tile_pool(name="acc", bufs=2) as apool,
            tc.tile_pool(name="ph0", bufs=3, space="PSUM") as ph0pool,
            tc.tile_pool(name="ph12", bufs=2, space="PSUM") as ph12pool,
            tc.tile_pool(name="po", bufs=2, space="PSUM") as popool,
            tc.tile_pool(name="pgate", bufs=1, space="PSUM") as pgpool,
        ):
            def load_xts(c, split=False):
                xts = xpool.tile([P, DK, NB], BF16, tag="xts", name=f"xts_{c}")
                if split:
                    # per-k-tile DMAs split over both HWDGE queues: the
                    # chunk-0 gate starts once all four 128KB tiles land
                    for dk in range(DK):
                        eng = nc.sync if dk < 2 else nc.scalar
                        eng.dma_start(
                            xts[:, dk, :],
                            xt_ap[dk * P:(dk + 1) * P, c * NB:(c + 1) * NB])
                else:
                    nc.sync.dma_start(
                        xts[:],
                        xt_ap[:, c * NB:(c + 1) * NB]
                        .rearrange("(dk p) b -> p dk b", p=P),
                    )
                return xts

            # Startup loads interleaved by first-use time across both HWDGE
            # queues.  w0 lives in four per-pair TILES: tile-granular DMA
            # dependencies would otherwise make L0(e0) wait for the whole
            # megabyte of w0.
            wg_sb = wpool.tile([P, DK, 32], BF16)
            nc.scalar.dma_start(wg_sb[:], wg_ap)
            xts_c = {0: load_xts(0, split=True)}
            w0p = [wpool.tile([P, 2, DK, DH], BF16, name=f"w0p{j}")
                   for j in range(4)]
            w1_sb = wpool.tile([P, E, DH], BF16)
            w2_sb = wpool.tile([P, E, DH], BF16)
            w3_sb = wpool.tile([P, E, DH], BF16)
            nc.scalar.dma_start(w0p[0][:], w0_ap[:, 0:2])
            nc.gpsimd.dma_start(w0p[1][:], w0_ap[:, 2:4])
            nc.scalar.dma_start(w1_sb[:], w1_ap)
            nc.gpsimd.dma_start(w3_sb[:], w3_ap)
            nc.scalar.dma_start(w2_sb[:], w2_ap)
            nc.gpsimd.dma_start(w0p[2][:], w0_ap[:, 4:6])
            nc.gpsimd.dma_start(w0p[3][:], w0_ap[:, 6:8])
            bg_sb = wpool.tile([32, 1], F32)
            nc.scalar.dma_start(bg_sb[:], bg_ap)
            xts_c[1] = load_xts(1, split=True)

            # HAM warmup: ~30 dummy matmuls on the (tiny, first-to-land) wg
            # tile into a scratch PSUM bank. PE activity from ~7us means the
            # clock throttle is at full rate when the real pipeline starts,
            # instead of spending its first ~10us at 1.2GHz.
            warm = popool.tile([32, P], F32, tag="po", name="warm")
            wgf = wg_sb[:].rearrange("p dk e -> p (dk e)")
            for _ in range(12):
                nc.tensor.matmul(warm[:], wg_sb[:, 0, :], wgf,
                                 start=True, stop=True)

            def emit_gate(xts_for, idx):
                # gate with 4x-replicated Wg columns: logitsT [32, b] where
                # partition q holds expert q%8 (same 512-cycle matmul as an
                # 8-wide gate); exp(z+bg) on ACT -> unnormalized ez, bf16
                pgt = pgpool.tile([32, NB], F32, tag="pgt", name=f"pgt_{idx}")
                for dk in range(DK):
                    nc.tensor.matmul(pgt[:], wg_sb[:, dk, :],
                                     xts_for[:, dk, :],
                                     start=(dk == 0), stop=(dk == DK - 1))
                ezT = spool.tile([32, NB], BF16, tag="ezT", name=f"ezT_{idx}")
                nc.scalar.activation(ezT[:], pgt[:],
                                     mybir.ActivationFunctionType.Exp,
                                     bias=bg_sb[:, 0:1])
                return ezT

            def emit_rs_compute(ezT, idx):
                # 1/sum_e(ez) in batch-partition layout: 32x32 DVE block
                # transpose puts batch%32 on partitions, a free-axis reduce
                # over the 8 experts (first replica) gives the sums, and the
                # reciprocal then costs only 16 elements per lane
                ezt = spool.tile([32, NB], BF16, tag="ezt", name=f"ezt_{idx}")
                nc.vector.transpose(ezt[:], ezT[:])
                sums = spool.tile([32, 16], F32, tag="sums",
                                  name=f"sums_{idx}")
                nc.vector.tensor_reduce(
                    sums[:],
                    ezt[:].rearrange("p (a b c) -> p a b c", b=4, c=8)
                    [:, :, 0, :],
                    axis=mybir.AxisListType.X, op=mybir.AluOpType.add)
                rs32 = spool.tile([32, 16], F32, tag="rs32",
                                  name=f"rs32_{idx}")
                nc.vector.reciprocal(rs32[:], sums[:])
                return rs32

            def emit_rs_spread(rs32, idx):
                # rs32[p32, 4*bb+g] -> rs128[g*32+p32, bb]: four DVE
                # crossbar quadrant copies regroup the sums to [128, BB]
                rs = spool.tile([P, BB], F32, tag="rs128", name=f"rs_{idx}")
                rv = rs32[:].rearrange("p (bb g) -> p bb g", g=4)
                ident = list(range(32))
                for g in range(4):
                    nc.vector.stream_shuffle(rs[g * 32:(g + 1) * 32, :],
                                             rv[:, :, g], ident)
                return rs

            ezbn = {}
            ezT_c = {}
            rs32_c = {}
            rs_c = {}

            def emit_bcast(c, e):
                # unnormalized ez row e -> partition-0 staging -> [128, NB]
                stg = stgpool.tile([1, NB], BF16, tag="stg",
                                   name=f"stg_{c}_{e}")
                nc.sync.dma_start(stg[:], ezT_c[c][e:e + 1, :])
                bz = bpool.tile([P, NB], BF16, tag="ezbn",
                                name=f"ezbn_{c}_{e}")
                nc.gpsimd.partition_broadcast(bz[:], stg[:])
                ezbn[(c, e)] = bz

            ezT_c[0] = emit_gate(xts_c[0], 0)
            rs32_c[0] = emit_rs_compute(ezT_c[0], 0)

            h0t, h1t, g2t = {}, {}, {}
            po_c = {}

            for s in range(S + 3):
                cq, r = divmod(s, 8)

                # ---- hoisted per-chunk events (gate, alpha, broadcasts,
                # next x chunk) keyed off the global stage ----
                if r == 0 and cq + 2 < CHUNKS:
                    xts_c[cq + 2] = load_xts(cq + 2)
                if cq == 0 and r < E and s < 8:
                    emit_bcast(0, r)
                # regroup this chunk's 1/sum to [128, BB] well after the
                # reciprocal finished (a dma_start whose source is pending
                # blocks the whole sync queue); chunk 0's waits until s==3
                if r == 1 and 1 <= cq < CHUNKS:
                    rs_c[cq] = emit_rs_spread(rs32_c.pop(cq), cq)
                if s == 3:
                    rs_c[0] = emit_rs_spread(rs32_c.pop(0), 0)
                if r == 5 and cq + 1 < CHUNKS:
                    ezT_c[cq + 1] = emit_gate(xts_c[cq + 1], cq + 1)
                if r == 6 and cq + 1 < CHUNKS:
                    rs32_c[cq + 1] = emit_rs_compute(ezT_c[cq + 1], cq + 1)
                if r == 7 and cq + 1 < CHUNKS:
                    emit_bcast(cq + 1, 0)
                    emit_bcast(cq + 1, 1)
                if r in (0, 1, 2) and 1 <= cq < CHUNKS:
                    emit_bcast(cq, 2 * r + 2)
                    emit_bcast(cq, 2 * r + 3)

                # ---- pipelined expert stages ----
                if s < S:                          # L0(stage s)
                    c, e = divmod(s, E)
                    ph0 = ph0pool.tile([P, NB], F32, tag="ph0")
                    for dk in range(DK):
                        nc.tensor.matmul(
                            ph0[:], w0p[e // 2][:, e % 2, dk, :],
                            xts_c[c][:, dk, :],
                            start=(dk == 0), stop=(dk == DK - 1))
                    h0t[s] = hpool.tile([P, NB], BF16, tag="h0",
                                        name=f"h0_{c}_{e}")
                    nc.scalar.activation(
                        h0t[s][:], ph0[:],
                        mybir.ActivationFunctionType.Relu)
                if 1 <= s + 1 - 1 and 0 <= s - 1 < S:  # L1(stage s-1)
                    c, e = divmod(s - 1, E)
                    ph1 = ph12pool.tile([P, NB], F32, tag="ph12")
                    nc.tensor.matmul(ph1[:], w1_sb[:, e, :], h0t[s - 1][:],
                                     start=True, stop=True)
                    h1t[s - 1] = hpool.tile([P, NB], BF16, tag="h1",
                                            name=f"h1_{c}_{e}")
                    if e < H1_ON_VECTOR:
                        nc.vector.tensor_scalar_max(h1t[s - 1][:], ph1[:],
                                                    0.0)
                    else:
                        nc.scalar.activation(
                            h1t[s - 1][:], ph1[:],
                            mybir.ActivationFunctionType.Relu)
                    del h0t[s - 1]
                    if e == E - 1:
                        del xts_c[c]
                if 0 <= s - 2 < S:                 # L2(stage s-2) + alpha
                    c, e = divmod(s - 2, E)
                    ph2 = ph12pool.tile([P, NB], F32, tag="ph12")
                    nc.tensor.matmul(ph2[:], w2_sb[:, e, :], h1t[s - 2][:],
                                     start=True, stop=True)
                    g2t[s - 2] = hpool.tile([P, NB], BF16, tag="g2",
                                            name=f"g2_{c}_{e}")
                    nc.vector.scalar_tensor_tensor(
                        g2t[s - 2][:], ph2[:], 0.0, ezbn[(c, e)][:],
                        mybir.AluOpType.max, mybir.AluOpType.mult)
                    del h1t[s - 2]
                    del ezbn[(c, e)]
                if 0 <= s - 3 < S:                 # L3(stage s-3), PSUM-acc
                    c, e = divmod(s - 3, E)
                    if e == 0:
                        po_c[c] = popool.tile([P, BB, DOUT], F32, tag="po",
                                              name=f"po_{c}")
                    for bb in range(BB):
                        # start=True clears has_written for the WHOLE bank,
                        # so only the first write may set it
                        nc.tensor.matmul(
                            po_c[c][:, bb, :],
                            g2t[s - 3][:, bb * P:(bb + 1) * P],
                            w3_sb[:, e, :],
                            start=(e == 0 and bb == 0),
                            stop=(e == E - 1))
                    del g2t[s - 3]
                    if e == E - 1:
                        # evict the expert-summed chunk output, normalizing
                        # by 1/sum(exp) (per-partition scalar), then store
                        acc = apool.tile([P, BB, DOUT], F32, tag="acc",
                                         name=f"acc_{c}")
                        yv = (y_ap[c * NB:(c + 1) * NB, :]
                              .rearrange("(bb p) o -> p bb o", p=P))
                        for bb in range(BB):
                            nc.vector.tensor_scalar_mul(
                                acc[:, bb, :], po_c[c][:, bb, :],
                                rs_c[c][:, bb:bb + 1])
                            nc.sync.dma_start(yv[:, bb:bb + 1, :],
                                              acc[:, bb:bb + 1, :])
                        del po_c[c]
                        del rs_c[c]

    nc.compile()
    _CACHE["nc"] = nc
    return nc


def _prep_inputs(x, Wg, bg, W0, W1, W2, W3):
    import ml_dtypes
    BF = ml_dtypes.bfloat16

    x = np.ascontiguousarray(np.asarray(x, dtype=np.float32))
    Wg = np.asarray(Wg, dtype=np.float32)
    bg = np.asarray(bg, dtype=np.float32)
    W0 = np.asarray(W0, dtype=np.float32)
    W1 = np.asarray(W1, dtype=np.float32)
    W2 = np.asarray(W2, dtype=np.float32)
    W3 = np.asarray(W3, dtype=np.float32)
    assert x.shape == (B, D)

    xt = np.ascontiguousarray(x.T.astype(BF))                       # [D, B]
    w0h = np.ascontiguousarray(
        W0.reshape(E, DK, P, DH).transpose(2, 0, 1, 3).astype(BF))
    w1h = np.ascontiguousarray(W1.transpose(1, 0, 2).astype(BF))
    w2h = np.ascontiguousarray(W2.transpose(1, 0, 2).astype(BF))
    w3h = np.ascontiguousarray(W3.transpose(1, 0, 2).astype(BF))
    wgh = np.ascontiguousarray(
        np.tile(Wg.reshape(DK, P, E), (1, 1, 4)).transpose(1, 0, 2)
        .astype(BF))
    bgh = np.ascontiguousarray(np.tile(bg.reshape(E, 1), (4, 1)))

    in_maps = []
    for core in range(N_CORES):
        sl = slice(core * B_LOCAL, (core + 1) * B_LOCAL)
        in_maps.append({
            "xt": np.ascontiguousarray(xt[:, sl]),
            "w0": w0h, "w1": w1h, "w2": w2h, "w3": w3h,
            "wg": wgh, "bg": bgh,
        })
    return in_maps


def _run(inputs, trace=False, **kwargs):
    nc = _build()
    in_maps = _prep_inputs(**inputs)
    res = run_bass_kernel_spmd(nc, in_maps, core_ids=list(range(N_CORES)),
                               trace=trace, **kwargs)
    y = np.concatenate([res.results[i]["y"] for i in range(N_CORES)], axis=0)
    return y, res


def kernel(**inputs):
    y, _ = _run(inputs)
    return y


# revision 51
# speedup vs baseline: 1.0134x; 1.0134x over previous
"""Batched MoE (dense routing) Trainium2 kernel, v3.

Reference computation (per batch row b):
    alpha = softmax(x @ Wg + bg)                      # (B, E)
    h = relu(x @ W0[e]); h = relu(h @ W1[e]); h = relu(h @ W2[e])
    h3[e] = h @ W3[e]
    y = sum_e alpha[:, e, None] * h3[e]               # (B, 128)

Shapes: B=65536, D=512, E=8, DH=128, DOUT=128.

Strategy: data-parallel shard B across 8 NeuronCores (8192 rows each);
weights replicated.  All matmul operands are bf16 (fp32 PSUM accumulate),
activations stay in transposed [feature, batch] layout so every MLP matmul
streams with free dim 512.

The alpha-weighted combine is folded into the pipeline algebraically:
alpha >= 0, so alpha*relu(z) = relu(alpha*z) and the scale can be applied
to any one layer.  The h2 relu eviction becomes a scalar_tensor_tensor
(relu, then multiply by the UNNORMALIZED exp(logit) row broadcast across
partitions by the otherwise-idle GPSIMD engine's partition_broadcast), and
the final layer ACCUMULATES over all 8 experts in PSUM (start=True only on
the bank's very first write -- it clears has_written for the whole bank).
The v1 per-expert combine (~180us of vector-engine time) disappears; one
scaled eviction per chunk stores the finished output.

Softmax normalization rides the output eviction: the gate matmul uses
4x-replicated Wg columns so exp(logits) lands on 32 partitions, a DVE 32x32
block transpose + free-axis reduce gives batch-partitioned sums, and a
cheap [32,16] reciprocal + 4 crossbar quadrant copies (stream_shuffle)
produce the [128, BB] per-partition 1/sum used by the eviction's
tensor_scalar_mul.  Nothing of this touches the PE or the scalar engine.

The expert loop is software-pipelined with a GLOBAL stage counter that
crosses chunk boundaries (stage s runs L0 of expert-stage s, L1 of s-1, L2
of s-2, L3 of s-3), so the in-order PE sees no bubble at chunk seams: the
next chunk's L0 matmuls separate the last expert's L2 from its L3.  h0/h1
relu evictions are split between the scalar and vector engines to balance
load.  Startup: weight/x DMAs are interleaved by first-use time across the
two HWDGE queues (sync + scalar), w0 lives in per-pair tiles so early
experts do not wait on the full megabyte (tile-granular DMA dependencies),
x is prefetched two chunks ahead, and a dozen dummy matmuls on the
first-to-land gate weights warm the HAM clock throttle before the real
pipeline starts.
"""

import numpy as np

import concourse.bass as bass
import concourse.tile as tile
from concourse import bacc, mybir
from concourse.bass_utils import run_bass_kernel_spmd

B, D, E, DH, DOUT = 65536, 512, 8, 128, 128
N_CORES = 8
B_LOCAL = B // N_CORES          # 8192
NB = 512                        # batch rows per chunk
CHUNKS = B_LOCAL // NB          # 16
DK = D // 128                   # 4 k-tiles over the input dim
P = 128
BB = NB // P                    # 4 batch sub-tiles per chunk
S = CHUNKS * E                  # 128 global expert-stages

F32 = mybir.dt.float32
F32R = mybir.dt.float32r
BF16 = mybir.dt.bfloat16

# how many of the 8 per-chunk h1 evictions go to the vector engine
# (the rest go to the scalar engine, which also has h0 + the gate exp)
H1_ON_VECTOR = 2

_CACHE = {}


def _build():
    if "nc" in _CACHE:
        return _CACHE["nc"]

    nc = bacc.Bacc("TRN2", target_bir_lowering=False, debug=False,
                   num_devices=N_CORES)

    xt_ap = nc.dram_tensor("xt", [D, B_LOCAL], BF16, kind="ExternalInput").ap()
    w0_ap = nc.dram_tensor("w0", [P, E, DK, DH], BF16, kind="ExternalInput").ap()
    w1_ap = nc.dram_tensor("w1", [P, E, DH], BF16, kind="ExternalInput").ap()
    w2_ap = nc.dram_tensor("w2", [P, E, DH], BF16, kind="ExternalInput").ap()
    w3_ap = nc.dram_tensor("w3", [P, E, DH], BF16, kind="ExternalInput").ap()
    wg_ap = nc.dram_tensor("wg", [P, DK, 32], BF16, kind="ExternalInput").ap()
    bg_ap = nc.dram_tensor("bg", [32, 1], F32, kind="ExternalInput").ap()
    y_ap = nc.dram_tensor("y", [B_LOCAL, DOUT], F32, kind="ExternalOutput").ap()

    with tile.TileContext(nc) as tc:
        with (
            tc.tile_pool(name="weights", bufs=1) as wpool,
            tc.tile_pool(name="xt", bufs=4) as xpool,
            tc.tile_pool(name="h", bufs=9) as hpool,
            tc.tile_pool(name="soft", bufs=3) as spool,
            tc.tile_pool(name="stg", bufs=4) as stgpool,
            tc.tile_pool(name="ezbn", bufs=14) as bpool,
            tc.tile_pool(name="acc", bufs=2) as apool,
            tc.tile_pool(name="ph0", bufs=2, space="PSUM") as ph0pool,
            tc.tile_pool(name="ph12", bufs=3, space="PSUM") as ph12pool,
            tc.tile_pool(name="po", bufs=2, space="PSUM") as popool,
            tc.tile_pool(name="pgate", bufs=1, space="PSUM") as pgpool,
        ):
            def load_xts(c, split=False):
                xts = xpool.tile([P, DK, NB], BF16, tag="xts", name=f"xts_{c}")
                if split:
                    # per-k-tile DMAs split over both HWDGE queues: the
                    # chunk-0 gate starts once all four 128KB tiles land
                    for dk in range(DK):
                        eng = nc.sync if dk < 2 else nc.scalar
                        eng.dma_start(
                            xts[:, dk, :],
                            xt_ap[dk * P:(dk + 1) * P, c * NB:(c + 1) * NB])
                else:
                    nc.sync.dma_start(
                        xts[:],
                        xt_ap[:, c * NB:(c + 1) * NB]
                        .rearrange("(dk p) b -> p dk b", p=P),
                    )
                return xts

            # Startup loads interleaved by first-use time across both HWDGE
            # queues.  w0 lives in four per-pair TILES: tile-granular DMA
            # dependencies would otherwise make L0(e0) wait for the whole
            # megabyte of w0.
            wg_sb = wpool.tile([P, DK, 32], BF16)
            nc.scalar.dma_start(wg_sb[:], wg_ap)
            xts_c = {0: load_xts(0, split=True)}
            w0p = [wpool.tile([P, 2, DK, DH], BF16, name=f"w0p{j}")
                   for j in range(4)]
            w1_sb = wpool.tile([P, E, DH], BF16)
            w2_sb = wpool.tile([P, E, DH], BF16)
            w3_sb = wpool.tile([P, E, DH], BF16)
            nc.scalar.dma_start(w0p[0][:], w0_ap[:, 0:2])
            nc.gpsimd.dma_start(w0p[1][:], w0_ap[:, 2:4])
            nc.scalar.dma_start(w1_sb[:], w1_ap)
            nc.gpsimd.dma_start(w3_sb[:], w3_ap)
            nc.scalar.dma_start(w2_sb[:], w2_ap)
            nc.gpsimd.dma_start(w0p[2][:], w0_ap[:, 4:6])
            nc.gpsimd.dma_start(w0p[3][:], w0_ap[:, 6:8])
            bg_sb = wpool.tile([32, 1], F32)
            nc.scalar.dma_start(bg_sb[:], bg_ap)
            xts_c[1] = load_xts(1, split=True)

            # HAM warmup: ~30 dummy matmuls on the (tiny, first-to-land) wg
            # tile into a scratch PSUM bank. PE activity from ~7us means the
            # clock throttle is at full rate when the real pipeline starts,
            # instead of spending its first ~10us at 1.2GHz.
            warm = popool.tile([32, P], F32, tag="po", name="warm")
            wgf = wg_sb[:].rearrange("p dk e -> p (dk e)")
            for _ in range(12):
                nc.tensor.matmul(warm[:], wg_sb[:, 0, :], wgf,
                                 start=True, stop=True)

            def emit_gate(xts_for, idx):
                # gate with 4x-replicated Wg columns: logitsT [32, b] where
                # partition q holds expert q%8 (same 512-cycle matmul as an
                # 8-wide gate); exp(z+bg) on ACT -> unnormalized ez, bf16
                pgt = pgpool.tile([32, NB], F32, tag="pgt", name=f"pgt_{idx}")
                for dk in range(DK):
                    nc.tensor.matmul(pgt[:], wg_sb[:, dk, :],
                                     xts_for[:, dk, :],
                                     start=(dk == 0), stop=(dk == DK - 1))
                ezT = spool.tile([32, NB], BF16, tag="ezT", name=f"ezT_{idx}")
                nc.scalar.activation(ezT[:], pgt[:],
                                     mybir.ActivationFunctionType.Exp,
                                     bias=bg_sb[:, 0:1])
                return ezT

            def emit_rs_compute(ezT, idx):
                # 1/sum_e(ez) in batch-partition layout: 32x32 DVE block
                # transpose puts batch%32 on partitions, a free-axis reduce
                # over the 8 experts (first replica) gives the sums, and the
                # reciprocal then costs only 16 elements per lane
                ezt = spool.tile([32, NB], BF16, tag="ezt", name=f"ezt_{idx}")
                nc.vector.transpose(ezt[:], ezT[:])
                sums = spool.tile([32, 16], F32, tag="sums",
                                  name=f"sums_{idx}")
                nc.vector.tensor_reduce(
                    sums[:],
                    ezt[:].rearrange("p (a b c) -> p a b c", b=4, c=8)
                    [:, :, 0, :],
                    axis=mybir.AxisListType.X, op=mybir.AluOpType.add)
                rs32 = spool.tile([32, 16], F32, tag="rs32",
                                  name=f"rs32_{idx}")
                nc.vector.reciprocal(rs32[:], sums[:])
                return rs32

            def emit_rs_spread(rs32, idx):
                # rs32[p32, 4*bb+g] -> rs128[g*32+p32, bb]: four DVE
                # crossbar quadrant copies regroup the sums to [128, BB]
                rs = spool.tile([P, BB], F32, tag="rs128", name=f"rs_{idx}")
                rv = rs32[:].rearrange("p (bb g) -> p bb g", g=4)
                ident = list(range(32))
                for g in range(4):
                    nc.vector.stream_shuffle(rs[g * 32:(g + 1) * 32, :],
                                             rv[:, :, g], ident)
                return rs

            ezbn = {}
            ezT_c = {}
            rs32_c = {}
            rs_c = {}

            def emit_bcast(c, e):
                # unnormalized ez row e -> partition-0 staging -> [128, NB]
                stg = stgpool.tile([1, NB], BF16, tag="stg",
                                   name=f"stg_{c}_{e}")
                nc.sync.dma_start(stg[:], ezT_c[c][e:e + 1, :])
                bz = bpool.tile([P, NB], BF16, tag="ezbn",
                                name=f"ezbn_{c}_{e}")
                nc.gpsimd.partition_broadcast(bz[:], stg[:])
                ezbn[(c, e)] = bz

            ezT_c[0] = emit_gate(xts_c[0], 0)
            rs32_c[0] = emit_rs_compute(ezT_c[0], 0)

            h0t, h1t, g2t = {}, {}, {}
            po_c = {}

            for s in range(S + 3):
                cq, r = divmod(s, 8)

                # ---- hoisted per-chunk events (gate, alpha, broadcasts,
                # next x chunk) keyed off the global stage ----
                if r == 0 and cq + 2 < CHUNKS:
                    xts_c[cq + 2] = load_xts(cq + 2)
                if cq == 0 and r < E and s < 8:
                    emit_bcast(0, r)
                # regroup this chunk's 1/sum to [128, BB] well after the
                # reciprocal finished (a dma_start whose source is pending
                # blocks the whole sync queue); chunk 0's waits until s==3
                if r == 1 and 1 <= cq < CHUNKS:
                    rs_c[cq] = emit_rs_spread(rs32_c.pop(cq), cq)
                if s == 3:
                    rs_c[0] = emit_rs_spread(rs32_c.pop(0), 0)
                if r == 5 and cq + 1 < CHUNKS:
                    ezT_c[cq + 1] = emit_gate(xts_c[cq + 1], cq + 1)
                if r == 6 and cq + 1 < CHUNKS:
                    rs32_c[cq + 1] = emit_rs_compute(ezT_c[cq + 1], cq + 1)
                if r == 7 and cq + 1 < CHUNKS:
                    emit_bcast(cq + 1, 0)
                    emit_bcast(cq + 1, 1)
                if r in (0, 1, 2) and 1 <= cq < CHUNKS:
                    emit_bcast(cq, 2 * r + 2)
                    emit_bcast(cq, 2 * r + 3)

                # ---- pipelined expert stages ----
                if s < S:                          # L0(stage s)
                    c, e = divmod(s, E)
                    ph0 = ph0pool.tile([P, NB], F32, tag="ph0")
                    for dk in range(DK):
                        nc.tensor.matmul(
                            ph0[:], w0p[e // 2][:, e % 2, dk, :],
                            xts_c[c][:, dk, :],
                            start=(dk == 0), stop=(dk == DK - 1))
                    h0t[s] = hpool.tile([P, NB], BF16, tag="h0",
                                        name=f"h0_{c}_{e}")
                    nc.scalar.activation(
                        h0t[s][:], ph0[:],
                        mybir.ActivationFunctionType.Relu)
                if 1 <= s + 1 - 1 and 0 <= s - 1 < S:  # L1(stage s-1)
                    c, e = divmod(s - 1, E)
                    ph1 = ph12pool.tile([P, NB], F32, tag="ph12")
                    nc.tensor.matmul(ph1[:], w1_sb[:, e, :], h0t[s - 1][:],
                                     start=True, stop=True)
                    h1t[s - 1] = hpool.tile([P, NB], BF16, tag="h1",
                                            name=f"h1_{c}_{e}")
                    if e < H1_ON_VECTOR:
                        nc.vector.tensor_scalar_max(h1t[s - 1][:], ph1[:],
                                                    0.0)
                    else:
                        nc.scalar.activation(
                            h1t[s - 1][:], ph1[:],
                            mybir.ActivationFunctionType.Relu)
                    del h0t[s - 1]
                    if e == E - 1:
                        del xts_c[c]
                if 0 <= s - 2 < S:                 # L2(stage s-2) + alpha
                    c, e = divmod(s - 2, E)
                    ph2 = ph12pool.tile([P, NB], F32, tag="ph12")
                    nc.tensor.matmul(ph2[:], w2_sb[:, e, :], h1t[s - 2][:],
                                     start=True, stop=True)
                    g2t[s - 2] = hpool.tile([P, NB], BF16, tag="g2",
                                            name=f"g2_{c}_{e}")
                    nc.vector.scalar_tensor_tensor(
                        g2t[s - 2][:], ph2[:], 0.0, ezbn[(c, e)][:],
                        mybir.AluOpType.max, mybir.AluOpType.mult)
                    del h1t[s - 2]
                    del ezbn[(c, e)]
                if 0 <= s - 3 < S:                 # L3(stage s-3), PSUM-acc
                    c, e = divmod(s - 3, E)
                    if e == 0:
                        po_c[c] = popool.tile([P, BB, DOUT], F32, tag="po",
                                              name=f"po_{c}")
                    for bb in range(BB):
                        # start=True clears has_written for the WHOLE bank,
                        # so only the first write may set it
                        nc.tensor.matmul(
                            po_c[c][:, bb, :],
                            g2t[s - 3][:, bb * P:(bb + 1) * P],
                            w3_sb[:, e, :],
                            start=(e == 0 and bb == 0),
                            stop=(e == E - 1))
                    del g2t[s - 3]
                    if e == E - 1:
                        # evict the expert-summed chunk output, normalizing
                        # by 1/sum(exp) (per-partition scalar), then store
                        acc = apool.tile([P, BB, DOUT], F32, tag="acc",
                                         name=f"acc_{c}")
                        yv = (y_ap[c * NB:(c + 1) * NB, :]
                              .rearrange("(bb p) o -> p bb o", p=P))
                        for bb in range(BB):
                            nc.vector.tensor_scalar_mul(
                                acc[:, bb, :], po_c[c][:, bb, :],
                                rs_c[c][:, bb:bb + 1])
                            nc.sync.dma_start(yv[:, bb:bb + 1, :],
                                              acc[:, bb:bb + 1, :])
                        del po_c[c]
                        del rs_c[c]

    nc.compile()
    _CACHE["nc"] = nc
    return nc


def _prep_inputs(x, Wg, bg, W0, W1, W2, W3):
    import ml_dtypes
    BF = ml_dtypes.bfloat16

    x = np.ascontiguousarray(np.asarray(x, dtype=np.float32))
    Wg = np.asarray(Wg, dtype=np.float32)
    bg = np.asarray(bg, dtype=np.float32)
    W0 = np.asarray(W0, dtype=np.float32)
    W1 = np.asarray(W1, dtype=np.float32)
    W2 = np.asarray(W2, dtype=np.float32)
    W3 = np.asarray(W3, dtype=np.float32)
    assert x.shape == (B, D)

    xt = np.ascontiguousarray(x.T.astype(BF))                       # [D, B]
    w0h = np.ascontiguousarray(
        W0.reshape(E, DK, P, DH).transpose(2, 0, 1, 3).astype(BF))
    w1h = np.ascontiguousarray(W1.transpose(1, 0, 2).astype(BF))
    w2h = np.ascontiguousarray(W2.transpose(1, 0, 2).astype(BF))
    w3h = np.ascontiguousarray(W3.transpose(1, 0, 2).astype(BF))
    wgh = np.ascontiguousarray(
        np.tile(Wg.reshape(DK, P, E), (1, 1, 4)).transpose(1, 0, 2)
        .astype(BF))
    bgh = np.ascontiguousarray(np.tile(bg.reshape(E, 1), (4, 1)))

    in_maps = []
    for core in range(N_CORES):
        sl = slice(core * B_LOCAL, (core + 1) * B_LOCAL)
        in_maps.append({
            "xt": np.ascontiguousarray(xt[:, sl]),
            "w0": w0h, "w1": w1h, "w2": w2h, "w3": w3h,
            "wg": wgh, "bg": bgh,
        })
    return in_maps


def _run(inputs, trace=False, **kwargs):
    nc = _build()
    in_maps = _prep_inputs(**inputs)
    res = run_bass_kernel_spmd(nc, in_maps, core_ids=list(range(N_CORES)),
                               trace=trace, **kwargs)
    y = np.concatenate([res.results[i]["y"] for i in range(N_CORES)], axis=0)
    return y, res


def kernel(**inputs):
    y, _ = _run(inputs)
    return y


# revision 52
# speedup vs baseline: 1.0217x; 1.0081x over previous
"""Batched MoE (dense routing) Trainium2 kernel, v3.

Reference computation (per batch row b):
    alpha = softmax(x @ Wg + bg)                      # (B, E)
    h = relu(x @ W0[e]); h = relu(h @ W1[e]); h = relu(h @ W2[e])
    h3[e] = h @ W3[e]
    y = sum_e alpha[:, e, None] * h3[e]               # (B, 128)

Shapes: B=65536, D=512, E=8, DH=128, DOUT=128.

Strategy: data-parallel shard B across 8 NeuronCores (8192 rows each);
weights replicated.  All matmul operands are bf16 (fp32 PSUM accumulate),
activations stay in transposed [feature, batch] layout so every MLP matmul
streams with free dim 512.

The alpha-weighted combine is folded into the pipeline algebraically:
alpha >= 0, so alpha*relu(z) = relu(alpha*z) and the scale can be applied
to any one layer.  The h2 relu eviction becomes a scalar_tensor_tensor
(relu, then multiply by the UNNORMALIZED exp(logit) row broadcast across
partitions by the otherwise-idle GPSIMD engine's partition_broadcast), and
the final layer ACCUMULATES over all 8 experts in PSUM (start=True only on
the bank's very first write -- it clears has_written for the whole bank).
The v1 per-expert combine (~180us of vector-engine time) disappears; one
scaled eviction per chunk stores the finished output.

Softmax normalization rides the output eviction: the gate matmul uses
4x-replicated Wg columns so exp(logits) lands on 32 partitions, a DVE 32x32
block transpose + free-axis reduce gives batch-partitioned sums, and a
cheap [32,16] reciprocal + 4 crossbar quadrant copies (stream_shuffle)
produce the [128, BB] per-partition 1/sum used by the eviction's
tensor_scalar_mul.  Nothing of this touches the PE or the scalar engine.

The expert loop is software-pipelined with a GLOBAL stage counter that
crosses chunk boundaries (stage s runs L0 of expert-stage s, L1 of s-1, L2
of s-2, L3 of s-3), so the in-order PE sees no bubble at chunk seams: the
next chunk's L0 matmuls separate the last expert's L2 from its L3.  h0/h1
relu evictions are split between the scalar and vector engines to balance
load.  Startup: weight/x DMAs are interleaved by first-use time across the
two HWDGE queues (sync + scalar), w0 lives in per-pair tiles so early
experts do not wait on the full megabyte (tile-granular DMA dependencies),
x is prefetched two chunks ahead, and a dozen dummy matmuls on the
first-to-land gate weights warm the HAM clock throttle before the real
pipeline starts.
"""

import numpy as np

import concourse.bass as bass
import concourse.tile as tile
from concourse import bacc, mybir
from concourse.bass_utils import run_bass_kernel_spmd

B, D, E, DH, DOUT = 65536, 512, 8, 128, 128
N_CORES = 8
B_LOCAL = B // N_CORES          # 8192
NB = 512                        # batch rows per chunk
CHUNKS = B_LOCAL // NB          # 16
DK = D // 128                   # 4 k-tiles over the input dim
P = 128
BB = NB // P                    # 4 batch sub-tiles per chunk
S = CHUNKS * E                  # 128 global expert-stages

F32 = mybir.dt.float32
F32R = mybir.dt.float32r
BF16 = mybir.dt.bfloat16

# how many of the 8 per-chunk h1 evictions go to the vector engine
# (the rest go to the scalar engine, which also has h0 + the gate exp)
H1_ON_VECTOR = 2

_CACHE = {}


def _build():
    if "nc" in _CACHE:
        return _CACHE["nc"]

    nc = bacc.Bacc("TRN2", target_bir_lowering=False, debug=False,
                   num_devices=N_CORES)

    xt_ap = nc.dram_tensor("xt", [D, B_LOCAL], BF16, kind="ExternalInput").ap()
    w0_ap = nc.dram_tensor("w0", [P, E, DK, DH], BF16, kind="ExternalInput").ap()
    w1_ap = nc.dram_tensor("w1", [P, E, DH], BF16, kind="ExternalInput").ap()
    w2_ap = nc.dram_tensor("w2", [P, E, DH], BF16, kind="ExternalInput").ap()
    w3_ap = nc.dram_tensor("w3", [P, E, DH], BF16, kind="ExternalInput").ap()
    wg_ap = nc.dram_tensor("wg", [P, DK, 32], BF16, kind="ExternalInput").ap()
    bg_ap = nc.dram_tensor("bg", [32, 1], F32, kind="ExternalInput").ap()
    y_ap = nc.dram_tensor("y", [B_LOCAL, DOUT], F32, kind="ExternalOutput").ap()

    with tile.TileContext(nc) as tc:
        with (
            tc.tile_pool(name="weights", bufs=1) as wpool,
            tc.tile_pool(name="xt", bufs=4) as xpool,
            tc.tile_pool(name="h", bufs=9) as hpool,
            tc.tile_pool(name="soft", bufs=3) as spool,
            tc.tile_pool(name="stg", bufs=4) as stgpool,
            tc.tile_pool(name="ezbn", bufs=14) as bpool,
            tc.tile_pool(name="acc", bufs=2) as apool,
            tc.tile_pool(name="ph0", bufs=2, space="PSUM") as ph0pool,
            tc.tile_pool(name="ph12", bufs=3, space="PSUM") as ph12pool,
            tc.tile_pool(name="po", bufs=2, space="PSUM") as popool,
            tc.tile_pool(name="pgate", bufs=1, space="PSUM") as pgpool,
        ):
            def load_xts(c, split=False):
                xts = xpool.tile([P, DK, NB], BF16, tag="xts", name=f"xts_{c}")
                if split:
                    # per-k-tile DMAs split over both HWDGE queues: the
                    # chunk-0 gate starts once all four 128KB tiles land
                    for dk in range(DK):
                        eng = nc.sync if dk < 2 else nc.scalar
                        eng.dma_start(
                            xts[:, dk, :],
                            xt_ap[dk * P:(dk + 1) * P, c * NB:(c + 1) * NB])
                else:
                    nc.sync.dma_start(
                        xts[:],
                        xt_ap[:, c * NB:(c + 1) * NB]
                        .rearrange("(dk p) b -> p dk b", p=P),
                    )
                return xts

            # Startup loads interleaved by first-use time across both HWDGE
            # queues.  w0 lives in four per-pair TILES: tile-granular DMA
            # dependencies would otherwise make L0(e0) wait for the whole
            # megabyte of w0.
            wg_sb = wpool.tile([P, DK, 32], BF16)
            nc.scalar.dma_start(wg_sb[:], wg_ap)
            xts_c = {0: load_xts(0, split=True)}
            w0p = [wpool.tile([P, 2, DK, DH], BF16, name=f"w0p{j}")
                   for j in range(4)]
            w1_sb = wpool.tile([P, E, DH], BF16)
            w2_sb = wpool.tile([P, E, DH], BF16)
            w3_sb = wpool.tile([P, E, DH], BF16)
            nc.scalar.dma_start(w0p[0][:], w0_ap[:, 0:2])
            nc.sync.dma_start(w0p[1][:], w0_ap[:, 2:4])
            nc.scalar.dma_start(w1_sb[:], w1_ap)
            nc.sync.dma_start(w3_sb[:], w3_ap)
            nc.scalar.dma_start(w2_sb[:], w2_ap)
            nc.sync.dma_start(w0p[2][:], w0_ap[:, 4:6])
            nc.scalar.dma_start(w0p[3][:], w0_ap[:, 6:8])
            bg_sb = wpool.tile([32, 1], F32)
            nc.scalar.dma_start(bg_sb[:], bg_ap)
            xts_c[1] = load_xts(1, split=True)

            # HAM warmup: ~30 dummy matmuls on the (tiny, first-to-land) wg
            # tile into a scratch PSUM bank. PE activity from ~7us means the
            # clock throttle is at full rate when the real pipeline starts,
            # instead of spending its first ~10us at 1.2GHz.
            warm = popool.tile([32, P], F32, tag="po", name="warm")
            wgf = wg_sb[:].rearrange("p dk e -> p (dk e)")
            for _ in range(12):
                nc.tensor.matmul(warm[:], wg_sb[:, 0, :], wgf,
                                 start=True, stop=True)

            def emit_gate(xts_for, idx):
                # gate with 4x-replicated Wg columns: logitsT [32, b] where
                # partition q holds expert q%8 (same 512-cycle matmul as an
                # 8-wide gate); exp(z+bg) on ACT -> unnormalized ez, bf16
                pgt = pgpool.tile([32, NB], F32, tag="pgt", name=f"pgt_{idx}")
                for dk in range(DK):
                    nc.tensor.matmul(pgt[:], wg_sb[:, dk, :],
                                     xts_for[:, dk, :],
                                     start=(dk == 0), stop=(dk == DK - 1))
                ezT = spool.tile([32, NB], BF16, tag="ezT", name=f"ezT_{idx}")
                nc.scalar.activation(ezT[:], pgt[:],
                                     mybir.ActivationFunctionType.Exp,
                                     bias=bg_sb[:, 0:1])
                return ezT

            def emit_rs_compute(ezT, idx):
                # 1/sum_e(ez) in batch-partition layout: 32x32 DVE block
                # transpose puts batch%32 on partitions, a free-axis reduce
                # over the 8 experts (first replica) gives the sums, and the
                # reciprocal then costs only 16 elements per lane
                ezt = spool.tile([32, NB], BF16, tag="ezt", name=f"ezt_{idx}")
                nc.vector.transpose(ezt[:], ezT[:])
                sums = spool.tile([32, 16], F32, tag="sums",
                                  name=f"sums_{idx}")
                nc.vector.tensor_reduce(
                    sums[:],
                    ezt[:].rearrange("p (a b c) -> p a b c", b=4, c=8)
                    [:, :, 0, :],
                    axis=mybir.AxisListType.X, op=mybir.AluOpType.add)
                rs32 = spool.tile([32, 16], F32, tag="rs32",
                                  name=f"rs32_{idx}")
                nc.vector.reciprocal(rs32[:], sums[:])
                return rs32

            def emit_rs_spread(rs32, idx):
                # rs32[p32, 4*bb+g] -> rs128[g*32+p32, bb]: four DVE
                # crossbar quadrant copies regroup the sums to [128, BB]
                rs = spool.tile([P, BB], F32, tag="rs128", name=f"rs_{idx}")
                rv = rs32[:].rearrange("p (bb g) -> p bb g", g=4)
                ident = list(range(32))
                for g in range(4):
                    nc.vector.stream_shuffle(rs[g * 32:(g + 1) * 32, :],
                                             rv[:, :, g], ident)
                return rs

            ezbn = {}
            ezT_c = {}
            rs32_c = {}
            rs_c = {}

            def emit_bcast(c, e):
                # unnormalized ez row e -> partition-0 staging -> [128, NB]
                stg = stgpool.tile([1, NB], BF16, tag="stg",
                                   name=f"stg_{c}_{e}")
                nc.sync.dma_start(stg[:], ezT_c[c][e:e + 1, :])
                bz = bpool.tile([P, NB], BF16, tag="ezbn",
                                name=f"ezbn_{c}_{e}")
                nc.gpsimd.partition_broadcast(bz[:], stg[:])
                ezbn[(c, e)] = bz

            ezT_c[0] = emit_gate(xts_c[0], 0)
            rs32_c[0] = emit_rs_compute(ezT_c[0], 0)

            h0t, h1t, g2t = {}, {}, {}
            po_c = {}

            for s in range(S + 3):
                cq, r = divmod(s, 8)

                # ---- hoisted per-chunk events (gate, alpha, broadcasts,
                # next x chunk) keyed off the global stage ----
                if r == 0 and cq + 2 < CHUNKS:
                    xts_c[cq + 2] = load_xts(cq + 2)
                if cq == 0 and r < E and s < 8:
                    emit_bcast(0, r)
                # regroup this chunk's 1/sum to [128, BB] well after the
                # reciprocal finished (a dma_start whose source is pending
                # blocks the whole sync queue); chunk 0's waits until s==3
                if r == 1 and 1 <= cq < CHUNKS:
                    rs_c[cq] = emit_rs_spread(rs32_c.pop(cq), cq)
                if s == 3:
                    rs_c[0] = emit_rs_spread(rs32_c.pop(0), 0)
                if r == 5 and cq + 1 < CHUNKS:
                    ezT_c[cq + 1] = emit_gate(xts_c[cq + 1], cq + 1)
                if r == 6 and cq + 1 < CHUNKS:
                    rs32_c[cq + 1] = emit_rs_compute(ezT_c[cq + 1], cq + 1)
                if r == 7 and cq + 1 < CHUNKS:
                    emit_bcast(cq + 1, 0)
                    emit_bcast(cq + 1, 1)
                if r in (0, 1, 2) and 1 <= cq < CHUNKS:
                    emit_bcast(cq, 2 * r + 2)
                    emit_bcast(cq, 2 * r + 3)

                # ---- pipelined expert stages ----
                if s < S:                          # L0(stage s)
                    c, e = divmod(s, E)
                    ph0 = ph0pool.tile([P, NB], F32, tag="ph0")
                    for dk in range(DK):
                        nc.tensor.matmul(
                            ph0[:], w0p[e // 2][:, e % 2, dk, :],
                            xts_c[c][:, dk, :],
                            start=(dk == 0), stop=(dk == DK - 1))
                    h0t[s] = hpool.tile([P, NB], BF16, tag="h0",
                                        name=f"h0_{c}_{e}")
                    nc.scalar.activation(
                        h0t[s][:], ph0[:],
                        mybir.ActivationFunctionType.Relu)
                if 1 <= s + 1 - 1 and 0 <= s - 1 < S:  # L1(stage s-1)
                    c, e = divmod(s - 1, E)
                    ph1 = ph12pool.tile([P, NB], F32, tag="ph12")
                    nc.tensor.matmul(ph1[:], w1_sb[:, e, :], h0t[s - 1][:],
                                     start=True, stop=True)
                    h1t[s - 1] = hpool.tile([P, NB], BF16, tag="h1",
                                            name=f"h1_{c}_{e}")
                    if e < H1_ON_VECTOR:
                        nc.vector.tensor_scalar_max(h1t[s - 1][:], ph1[:],
                                                    0.0)
                    else:
                        nc.scalar.activation(
                            h1t[s - 1][:], ph1[:],
                            mybir.ActivationFunctionType.Relu)
                    del h0t[s - 1]
                    if e == E - 1:
                        del xts_c[c]
                if 0 <= s - 2 < S:                 # L2(stage s-2) + alpha
                    c, e = divmod(s - 2, E)
                    ph2 = ph12pool.tile([P, NB], F32, tag="ph12")
                    nc.tensor.matmul(ph2[:], w2_sb[:, e, :], h1t[s - 2][:],
                                     start=True, stop=True)
                    g2t[s - 2] = hpool.tile([P, NB], BF16, tag="g2",
                                            name=f"g2_{c}_{e}")
                    nc.vector.scalar_tensor_tensor(
                        g2t[s - 2][:], ph2[:], 0.0, ezbn[(c, e)][:],
                        mybir.AluOpType.max, mybir.AluOpType.mult)
                    del h1t[s - 2]
                    del ezbn[(c, e)]
                if 0 <= s - 3 < S:                 # L3(stage s-3), PSUM-acc
                    c, e = divmod(s - 3, E)
                    if e == 0:
                        po_c[c] = popool.tile([P, BB, DOUT], F32, tag="po",
                                              name=f"po_{c}")
                    for bb in range(BB):
                        # start=True clears has_written for the WHOLE bank,
                        # so only the first write may set it
                        nc.tensor.matmul(
                            po_c[c][:, bb, :],
                            g2t[s - 3][:, bb * P:(bb + 1) * P],
                            w3_sb[:, e, :],
                            start=(e == 0 and bb == 0),
                            stop=(e == E - 1))
                    del g2t[s - 3]
                    if e == E - 1:
                        # evict the expert-summed chunk output, normalizing
                        # by 1/sum(exp) (per-partition scalar), then store
                        acc = apool.tile([P, BB, DOUT], F32, tag="acc",
                                         name=f"acc_{c}")
                        yv = (y_ap[c * NB:(c + 1) * NB, :]
                              .rearrange("(bb p) o -> p bb o", p=P))
                        for bb in range(BB):
                            nc.vector.tensor_scalar_mul(
                                acc[:, bb, :], po_c[c][:, bb, :],
                                rs_c[c][:, bb:bb + 1])
                            nc.sync.dma_start(yv[:, bb:bb + 1, :],
                                              acc[:, bb:bb + 1, :])
                        del po_c[c]
                        del rs_c[c]

    nc.compile()
    _CACHE["nc"] = nc
    return nc


def _prep_inputs(x, Wg, bg, W0, W1, W2, W3):
    import ml_dtypes
    BF = ml_dtypes.bfloat16

    x = np.ascontiguousarray(np.asarray(x, dtype=np.float32))
    Wg = np.asarray(Wg, dtype=np.float32)
    bg = np.asarray(bg, dtype=np.float32)
    W0 = np.asarray(W0, dtype=np.float32)
    W1 = np.asarray(W1, dtype=np.float32)
    W2 = np.asarray(W2, dtype=np.float32)
    W3 = np.asarray(W3, dtype=np.float32)
    assert x.shape == (B, D)

    xt = np.ascontiguousarray(x.T.astype(BF))                       # [D, B]
    w0h = np.ascontiguousarray(
        W0.reshape(E, DK, P, DH).transpose(2, 0, 1, 3).astype(BF))
    w1h = np.ascontiguousarray(W1.transpose(1, 0, 2).astype(BF))
    w2h = np.ascontiguousarray(W2.transpose(1, 0, 2).astype(BF))
    w3h = np.ascontiguousarray(W3.transpose(1, 0, 2).astype(BF))
    wgh = np.ascontiguousarray(
        np.tile(Wg.reshape(DK, P, E), (1, 1, 4)).transpose(1, 0, 2)
        .astype(BF))
    bgh = np.ascontiguousarray(np.tile(bg.reshape(E, 1), (4, 1)))

    in_maps = []
    for core in range(N_CORES):
        sl = slice(core * B_LOCAL, (core + 1) * B_LOCAL)
        in_maps.append({
            "xt": np.ascontiguousarray(xt[:, sl]),
            "w0": w0h, "w1": w1h, "w2": w2h, "w3": w3h,
            "wg": wgh, "bg": bgh,
        })
    return in_maps


def _run(inputs, trace=False, **kwargs):
    nc = _build()
    in_maps = _prep_inputs(**inputs)
    res = run_bass_kernel_spmd(nc, in_maps, core_ids=list(range(N_CORES)),
                               trace=trace, **kwargs)
    y = np.concatenate([res.results[i]["y"] for i in range(N_CORES)], axis=0)
    return y, res


def kernel(**inputs):
    y, _ = _run(inputs)
    return y
